# revision 24
# baseline (speedup 1.0000x reference)
"""Binarized 4-layer MLP (8192x784 -> 6144 -> 6144 -> 6144 -> 10, log_softmax)
on 8 Trainium2 NeuronCores, data-parallel over the batch.

Per-core dataflow (batch slice of 1024, feature-major activations [feat, batch]):
  fc1: x @ sign(w1).T as a 2-term fp16 hi/lo split of x, with the two terms
       stacked along the contraction dim (1568 rows -> 13 k-tiles). fp16
       upconverts losslessly to the PE's e10m11 internal format and the
       weights are exactly +-1, so this reproduces fp32 accuracy.
  fc2/fc3: sign(h) @ sign(w).T in fp8e4 with DoubleRow perf mode. All products
       are +-1 and partial sums are small integers, so fp32 PSUM accumulation
       is bit-exact regardless of order.
  fc4: fused into the fc3 m-loop, 4-way column-tiled (each m-tile's 10-wide
       w4 block, zero-padded to 32 columns, lands on a distinct 32-column
       strip of the PE array, so 4 matmuls run concurrently). b4/4 is folded
       into the PSUM->SBUF copy bias so summing the 4 strips also applies b4.
  log_softmax: per 128-batch tile, one full 128x128 PE transpose, then the 4
       class strips are summed with narrow DVE adds; exp/sum/ln batched into
       single ACT/DVE ops. Logits are O(4) so the max-subtract is dropped
       (exp cannot overflow; result is mathematically identical).
"""

import numpy as np
import ml_dtypes

import concourse.bass as bass
import concourse.mybir as mybir
from concourse import bacc
from concourse.tile import TileContext
from concourse.bass_utils import run_bass_kernel_spmd
from concourse.masks import make_identity

dt = mybir.dt

CORES = 8
B = 8192
BC = B // CORES          # 1024 batch rows per core
DIN = 784
KT1 = 13                 # fc1 contraction tiles: 2*784 = 1568 padded to 1664
K1P = KT1 * 128
DH = 6144
MT = DH // 128           # 48 feature tiles
KB = DH // 256           # 24 DoubleRow contraction blocks
DOUT = 10
NH = BC // 512           # 2 moving halves of 512
MQ = 12                  # fc1 m-groups (w1 streamed per 4 m-tiles)
MPQ = MT // MQ
NJ = BC // 128           # 8 output j-tiles

FP8 = mybir.dt.np(dt.float8e4)

last_exec_time_ns = None


def _build_program():
    nc = bacc.Bacc("TRN2", target_bir_lowering=False, debug=False,
                   num_devices=CORES)

    xt = nc.dram_tensor("xt", [128, KT1, BC], dt.float16,
                        kind="ExternalInput").ap()
    w1t = nc.dram_tensor("w1t", [MQ, 128, KT1, MPQ * 128], dt.float16,
                         kind="ExternalInput").ap()
    w2p = nc.dram_tensor("w2p", [MT, 128, KB, 2, 128], dt.float8e4,
                         kind="ExternalInput").ap()
    w3p = nc.dram_tensor("w3p", [MT, 128, KB, 2, 128], dt.float8e4,
                         kind="ExternalInput").ap()
    w4p = nc.dram_tensor("w4p", [128, MT, 32], dt.float16,
                         kind="ExternalInput").ap()
    b1p = nc.dram_tensor("b1p", [128, MT], dt.float32, kind="ExternalInput").ap()
    b2p = nc.dram_tensor("b2p", [128, MT], dt.float32, kind="ExternalInput").ap()
    b3p = nc.dram_tensor("b3p", [128, MT], dt.float32, kind="ExternalInput").ap()
    b4q = nc.dram_tensor("b4q", [128, 1], dt.float32, kind="ExternalInput").ap()
    out = nc.dram_tensor("out", [BC, DOUT], dt.float32, kind="ExternalOutput").ap()

    DR = mybir.MatmulPerfMode.DoubleRow
    AF = mybir.ActivationFunctionType
    ALU = mybir.AluOpType

    with TileContext(nc) as tc:
        with tc.tile_pool(name="consts", bufs=1) as cpool, \
             tc.tile_pool(name="h2p", bufs=1) as h2pool, \
             tc.tile_pool(name="lgp", bufs=1, space="PSUM") as lgp:
            w2f = cpool.tile([128, KB, 2, 128], dt.float8e4)
            w3f = cpool.tile([128, KB, 2, 128], dt.float8e4)
            b1_sb = cpool.tile([128, MT], dt.float32)
            b2_sb = cpool.tile([128, MT], dt.float32)
            b3_sb = cpool.tile([128, MT], dt.float32)
            b4_sb = cpool.tile([128, 1], dt.float32)
            w4_sb = cpool.tile([128, MT, 32], dt.float16)
            ident = cpool.tile([128, 128], dt.float32)
            warm_w = cpool.tile([128, 64], dt.float16)

            # fc4 logits accumulate here; two tiles (one per 512-batch half /
            # PSUM bank) so the tail can start on half A while half B finishes
            lgA = lgp.tile([128, 512], dt.float32)
            lgB = lgp.tile([128, 512], dt.float32)
            lgs = [lgA, lgB]

            with tc.tile_pool(name="ps", bufs=3, space="PSUM") as pspool:
                with tc.tile_pool(name="h1p", bufs=1) as h1pool:
                    with tc.tile_pool(name="x1p", bufs=1) as x1pool:
                        # --- startup DMAs in fc1 consumption order
                        # (k-interleaved), alternating dispatch engines ---
                        # warm up the PE HAM clock gate while the first x/w1
                        # DMAs are in flight. The memset runs on the Vector
                        # engine (empty queue) so the warm matmuls start
                        # immediately instead of sitting behind the GpSimd
                        # DMA dispatches.
                        nc.vector.memset(warm_w[:], 0.0)
                        for _ in range(20):
                            nc.tensor.matmul(lgA[0:64, 0:64], warm_w[:],
                                             warm_w[:], start=True, stop=True)

                        w1q0 = {}
                        xt_half = {}
                        for k in range(KT1):
                            if k == 0:
                                for n in range(NH):
                                    tx = x1pool.tile([128, 512], dt.float16,
                                                     tag=f"xt0_{n}")
                                    nc.sync.dma_start(
                                        out=tx[:],
                                        in_=xt[:, 0, n * 512:(n + 1) * 512])
                                    xt_half[(0, n)] = tx[:, :]
                            else:
                                tx = x1pool.tile([128, BC], dt.float16,
                                                 tag=f"xt_{k}")
                                nc.sync.dma_start(out=tx[:], in_=xt[:, k, :])
                                for n in range(NH):
                                    xt_half[(k, n)] = tx[:, n * 512:(n + 1) * 512]
                            tw = x1pool.tile([128, MPQ * 128], dt.float16,
                                             tag=f"w1q0_{k}")
                            nc.gpsimd.dma_start(out=tw[:], in_=w1t[0, :, k, :])
                            w1q0[k] = tw
                        nc.sync.dma_start(out=w2f[:], in_=w2p[0])
                        nc.gpsimd.dma_start(out=w3f[:], in_=w3p[0])
                        nc.sync.dma_start(out=b1_sb[:], in_=b1p[:])
                        nc.gpsimd.dma_start(out=b2_sb[:], in_=b2p[:])
                        nc.sync.dma_start(out=b3_sb[:], in_=b3p[:])
                        nc.gpsimd.dma_start(out=b4_sb[:], in_=b4q[:])
                        nc.sync.dma_start(out=w4_sb[:], in_=w4p[:])
                        make_identity(nc, ident[:])
                        # pre-warm the Exp/Ln activation tables so the
                        # log_softmax tail doesn't pay ACT_TABLE_LOADs serially
                        warm = cpool.tile([1, 1], dt.float32)
                        nc.scalar.activation(warm[:], ident[0:1, 0:1], AF.Exp)
                        nc.scalar.activation(warm[:], warm[:], AF.Ln)

                        # h1/h2 split in halves so cross-phase waits resolve
                        # earlier than the last sign() of the previous phase
                        h1a = h1pool.tile([128, MT // 2, BC], dt.float8e4)
                        h1b = h1pool.tile([128, MT // 2, BC], dt.float8e4)

                        def h1_of(m):
                            return (h1a if m < MT // 2 else h1b)[
                                :, m % (MT // 2), :]

                        def h1_pair(b, n):
                            t = h1a if 2 * b < MT // 2 else h1b
                            mb = (2 * b) % (MT // 2)
                            return t[:, mb:mb + 2, n * 512:(n + 1) * 512]

                        # ---------------- fc1 ----------------
                        with tc.tile_pool(name="w1pool", bufs=3) as w1pool:
                            for q in range(MQ):
                                if q == 0:
                                    def lhs1(k, mi):
                                        return w1q0[k][:, mi * 128:(mi + 1) * 128]
                                else:
                                    w1q = w1pool.tile([128, KT1, MPQ * 128],
                                                      dt.float16, tag="w1")
                                    nc.sync.dma_start(out=w1q[:], in_=w1t[q])

                                    def lhs1(k, mi, w1q=w1q):
                                        return w1q[:, k, mi * 128:(mi + 1) * 128]
                                for mi in range(MPQ):
                                    m = q * MPQ + mi
                                    psum = pspool.tile([128, BC], dt.float32,
                                                       tag="ps")
                                    # m=0: mark sub-group stops so the Tile
                                    # scheduler's group-leader wait coalescing
                                    # only makes the first matmul wait for the
                                    # first few k-tile DMAs, not all 13
                                    stops = ({2, 5, 8, KT1 - 1} if m == 0
                                             else {KT1 - 1})
                                    for k in range(KT1):
                                        for n in range(NH):
                                            nc.tensor.matmul(
                                                psum[:, n * 512:(n + 1) * 512],
                                                lhs1(k, mi),
                                                xt_half[(k, n)],
                                                start=(k == 0),
                                                stop=(k in stops),
                                                skip_group_check=True,
                                            )
                                    nc.scalar.sign(h1_of(m), psum[:, :],
                                                   bias=b1_sb[:, m:m + 1])

                    # ---------------- fc2 ----------------
                    h2a = h2pool.tile([128, MT // 2, BC], dt.float8e4)
                    h2b = h2pool.tile([128, MT // 2, BC], dt.float8e4)

                    def h2_of(m):
                        return (h2a if m < MT // 2 else h2b)[:, m % (MT // 2), :]

                    def h2_pair(b, n):
                        t = h2a if 2 * b < MT // 2 else h2b
                        mb = (2 * b) % (MT // 2)
                        return t[:, mb:mb + 2, n * 512:(n + 1) * 512]

                    with tc.tile_pool(name="w2pool", bufs=3) as w2pool:
                        for m in range(MT):
                            if m == 0:
                                wsb = w2f
                            else:
                                wsb = w2pool.tile([128, KB, 2, 128],
                                                  dt.float8e4, tag="w2")
                                nc.sync.dma_start(out=wsb[:], in_=w2p[m])
                            psum = pspool.tile([128, BC], dt.float32, tag="ps")
                            # b-outer: both n-halves consume the same
                            # stationary back-to-back (smaller effective
                            # LDWEIGHTS traffic / instruction stream)
                            for b in range(KB):
                                for n in range(NH):
                                    nc.tensor.matmul(
                                        psum[:, n * 512:(n + 1) * 512],
                                        wsb[:, b],
                                        h1_pair(b, n),
                                        start=(b == 0),
                                        stop=(b == KB - 1),
                                        perf_mode=DR,
                                        skip_group_check=True,
                                    )
                            nc.scalar.sign(h2_of(m), psum[:, :],
                                           bias=b2_sb[:, m:m + 1])

                # ---------------- fc3 + fused fc4 ----------------
                with tc.tile_pool(name="w3pool", bufs=3) as w3pool, \
                     tc.tile_pool(name="h3pool", bufs=MT) as h3pool:
                    h3_tiles = [None] * MT

                    def fc4_mm(m, n):
                        # 4-way column tiling: m-tile m lands on col strip
                        # 32q..32q+31 (w4 zero-padded to 32 cols). start=True
                        # clears has_written only for the WRITTEN partitions
                        # of the bank, so each strip's first matmul (the whole
                        # first quad, m=0..3) must carry start=True — one
                        # shared start would leave the other strips
                        # accumulating onto stale PSUM state from a previous
                        # execution.
                        q = m % 4
                        nc.tensor.matmul(
                            lgs[n][32 * q:32 * q + 32, :],
                            w4_sb[:, m, :],
                            h3_tiles[m][:, n * 512:(n + 1) * 512],
                            start=(m < 4),
                            stop=(m >= MT - 4),
                            tile_position=(0, 32 * q),
                            skip_group_check=True,
                        )

                    for m in range(MT):
                        if m == 0:
                            wsb = w3f
                        else:
                            wsb = w3pool.tile([128, KB, 2, 128],
                                              dt.float8e4, tag="w3")
                            nc.sync.dma_start(out=wsb[:], in_=w3p[m])
                        psum = pspool.tile([128, BC], dt.float32, tag="ps")
                        for b in range(KB):
                            for n in range(NH):
                                nc.tensor.matmul(
                                    psum[:, n * 512:(n + 1) * 512],
                                    wsb[:, b],
                                    h2_pair(b, n),
                                    start=(b == 0),
                                    stop=(b == KB - 1),
                                    perf_mode=DR,
                                    skip_group_check=True,
                                )
                        t_h3 = h3pool.tile([128, BC], dt.float16, tag="h3")
                        nc.scalar.activation(t_h3[:], psum[:, :],
                                             AF.Identity,
                                             bias=b3_sb[:, m:m + 1])
                        nc.vector.tensor_scalar(t_h3[:], t_h3[:], 1.0, -1.0,
                                                ALU.min, ALU.max)
                        h3_tiles[m] = t_h3
                    # all of fc4 in one n-major block after the DoubleRow
                    # stream: avoids the ~310ns full-array drain transition
                    # each interleaved quad batch cost, and half A's logits
                    # complete a full half-block early so the tail's ACT/
                    # transpose work on half A overlaps half B's matmuls
                    for n in range(NH):
                        for mm in range(MT):
                            fc4_mm(mm, n)

            # ---------------- log_softmax tail ----------------
            # b4/4 rides in as the ACT bias here (each class appears in 4
            # strips, so the quarters sum back to b4)
            with tc.tile_pool(name="sm", bufs=1) as smp:
                # per-half tiles so half A's transposes don't wait on the
                # half-B copy (n-major fc4 flush finishes bank A first)
                lgsbA = smp.tile([128, 512], dt.float32, tag="lgsbA")
                lgsbB = smp.tile([128, 512], dt.float32, tag="lgsbB")
                lgsb_h = [lgsbA, lgsbB]
                for n in range(NH):
                    nc.scalar.activation(lgsb_h[n][:], lgs[n][:],
                                         AF.Identity, bias=b4_sb[:, 0:1])
                lgall = smp.tile([128, NJ, DOUT], dt.float32, tag="lgall")
                s01 = smp.tile([128, NJ, DOUT], dt.float32, tag="s01")
                s2c = smp.tile([128, NJ, DOUT], dt.float32, tag="s2c")
                with tc.tile_pool(name="tp", bufs=2, space="PSUM") as tpp:
                    for j in range(NJ):
                        # full-bank tile: the ping-pong buffers must land in
                        # different PSUM banks, else the PE transpose write
                        # races the DVE strip reads of the previous j
                        # (PE-W + DVE-R same bank is undefined)
                        tpw = tpp.tile([128, 512], dt.float32, tag=f"tp{j % 2}")
                        tp = tpw[:, 0:128]
                        nc.tensor.transpose(
                            tp,
                            lgsb_h[j // 4][:, (j % 4) * 128:(j % 4 + 1) * 128],
                            ident[:])
                        # sum the 4 class strips (now along the free dim);
                        # DVE two-source ops can't read PSUM twice, so copy
                        # one strip per pair to SBUF first
                        nc.vector.tensor_copy(s01[:, j, :], tp[:, 0:DOUT])
                        nc.vector.tensor_copy(s2c[:, j, :], tp[:, 64:64 + DOUT])
                        nc.vector.scalar_tensor_tensor(
                            s01[:, j, :], tp[:, 32:32 + DOUT], 0.0,
                            s01[:, j, :], ALU.add, ALU.add)
                        nc.vector.scalar_tensor_tensor(
                            s2c[:, j, :], tp[:, 96:96 + DOUT], 0.0,
                            s2c[:, j, :], ALU.add, ALU.add)
                        nc.vector.scalar_tensor_tensor(
                            lgall[:, j, :], s01[:, j, :], 0.0,
                            s2c[:, j, :], ALU.add, ALU.add)
                    # logits are O(4): exp cannot overflow, skip max-subtract
                    ex = smp.tile([128, NJ, DOUT], dt.float32, tag="ex")
                    nc.scalar.activation(ex[:], lgall[:], AF.Exp)
                    sums = smp.tile([128, NJ], dt.float32, tag="sums")
                    nc.vector.tensor_reduce(sums[:], ex[:],
                                            mybir.AxisListType.X, ALU.add)
                    lns = smp.tile([128, NJ], dt.float32, tag="lns")
                    nc.scalar.activation(lns[:], sums[:], AF.Ln)
                    res = smp.tile([128, NJ, DOUT], dt.float32, tag="res")
                    for j in range(NJ):
                        nc.vector.tensor_scalar(res[:, j, :], lgall[:, j, :],
                                                lns[:, j:j + 1], None,
                                                ALU.subtract)
                        # alternate dispatch queues: 8 back-to-back DMAs
                        # serialize ~0.7us each on a single queue
                        eng = nc.sync if j % 2 == 0 else nc.gpsimd
                        eng.dma_start(
                            out=out[j * 128:(j + 1) * 128, :],
                            in_=res[:, j, :])

    nc.compile()
    return nc


def _pack_inputs(x, w1, b1, w2, b2, w3, b3, w4, b4):
    """Host-side packing into the device layouts. Shared tensors are packed
    once; only xt differs per core."""
    f32 = np.float32
    f16 = np.float16
    x = np.asarray(x, f32).reshape(B, DIN)

    # fc1 weights: sign(w1).T stacked twice (hi/lo terms share the weights),
    # padded to [1664, 6144], layout [q, p, k, m]
    s1 = np.sign(np.asarray(w1, f32))                       # [DH, DIN]
    s1t = np.zeros((K1P, DH), f16)
    s1t[:DIN] = s1.T
    s1t[DIN:2 * DIN] = s1.T
    w1t = np.ascontiguousarray(
        s1t.reshape(KT1, 128, MQ, MPQ * 128).transpose(2, 1, 0, 3))

    def pack_dr(w):
        # sign(w).T -> [mo, p, b, i, m'] DoubleRow stationary layout
        st = np.sign(np.asarray(w, f32)).T                  # [in, out]
        r = st.reshape(KB, 2, 128, MT, 128)                 # [b, i, p, mo, m']
        return np.ascontiguousarray(r.transpose(3, 2, 0, 1, 4)).astype(FP8)

    w2p = pack_dr(w2)
    w3p = pack_dr(w3)

    # fc4 weights: w4.T in fp16, layout [p, j, c] zero-padded 10 -> 32 cols
    w4t = np.asarray(w4, f32).T.astype(f16)                 # [DH, DOUT]
    w4p = np.zeros((128, MT, 32), f16)
    w4p[:, :, :DOUT] = w4t.reshape(MT, 128, DOUT).transpose(1, 0, 2)
    w4p = np.ascontiguousarray(w4p)

    def pack_b(b):
        return np.ascontiguousarray(np.asarray(b, f32).reshape(MT, 128).T)

    b1p, b2p, b3p = pack_b(b1), pack_b(b2), pack_b(b3)
    # b4/4 replicated on each 32-partition strip (the 4 strips sum to b4)
    b4v = np.asarray(b4, f32).reshape(DOUT)
    b4q = np.zeros((128, 1), f32)
    for q in range(4):
        b4q[32 * q:32 * q + DOUT, 0] = b4v / 4.0

    shared = {"w1t": w1t, "w2p": w2p, "w3p": w3p, "w4p": w4p,
              "b1p": b1p, "b2p": b2p, "b3p": b3p, "b4q": b4q}

    # per-core x: fp16 hi/lo split stacked along contraction, layout [p, k, n]
    in_maps = []
    for c in range(CORES):
        xc = x[c * BC:(c + 1) * BC]                         # [BC, DIN]
        hi = xc.astype(f16)
        lo = (xc - hi.astype(f32)).astype(f16)
        arr = np.zeros((K1P, BC), f16)
        arr[:DIN] = hi.T
        arr[DIN:2 * DIN] = lo.T
        xt = np.ascontiguousarray(arr.reshape(KT1, 128, BC).transpose(1, 0, 2))
        in_maps.append({"xt": xt, **shared})
    return in_maps


_cached_nc = None


def kernel(x, w1, b1, w2, b2, w3, b3, w4, b4):
    global _cached_nc, last_exec_time_ns
    import os
    trace = bool(int(os.environ.get("KERNEL_TRACE", "0")))
    if _cached_nc is None:
        _cached_nc = _build_program()
    in_maps = _pack_inputs(x, w1, b1, w2, b2, w3, b3, w4, b4)
    res = run_bass_kernel_spmd(_cached_nc, in_maps, list(range(CORES)),
                               trace=trace)
    last_exec_time_ns = res.exec_time_ns
    return np.concatenate([res.results[c]["out"] for c in range(CORES)], axis=0)
